# revision 7
# baseline (speedup 1.0000x reference)
"""DSoftKI Trainium2 kernel (v2).

Reference computation (per batch row b, interp point m, dim d; B=16384, M=512, D=8):
    diff[b,m,d] = x[b,d]/T[m,d] - z[m,d]
    dist[b,m]   = ||diff[b,m,:]||
    W           = softmax_m(-dist)
    dd          = diff / (dist+1e-6) / T          = numer * R
    mean_dd[b,d]= sum_m W*dd                      (acc)
    deriv       = -W*(dd - mean_dd)               = acc_d*W - V*numer_d
    out         = concat([W  (B rows) , deriv transposed to (b*D+d, m) rows])

with numer[b,m,d] = x[b,d]*A[m,d] - Bz[m,d],  A = 1/T^2, Bz = z/T,
     R = 1/dist, U = exp(-dist), S = sum_m U, W = U/S, V = W*R.

v2 strategy vs v1: every tensor that legally can be bf16 is bf16 so the DVE
runs in its 2x (tensor_tensor) / 4x (tensor_scalar) perf modes, outputs ship
as bf16 (host upcasts; tolerance is 2e-2), the ACT chain is cut from 5+7
passes to 4 (Ln, d2, U+S-accum, R) with W = U*(1/S) on DVE 4x and V = W*R on
DVE 2x, and the 8 G-passes / 8 deriv-planes are load-balanced across
DVE / GPSIMD / ACT via the assignment tables below.

Sharding: data-parallel over B across 8 cores (2048 rows each); z/T-derived
constants replicated.  Host precomputes all x-transposed/split operands, so
the device program needs no transposes and no broadcasts.
"""
import sys

sys.path.insert(0, "/opt/trn_rl_repo")

import numpy as np
import ml_dtypes

import concourse.bass as bass
import concourse.tile as tile
from concourse import bacc, mybir
from concourse.bass_utils import run_bass_kernel_spmd

dt = mybir.dt
AF = mybir.ActivationFunctionType
OP = mybir.AluOpType

B, M, D = 16384, 512, 8
N_CORES = 8
BSH = B // N_CORES          # 2048 rows per core
NT = BSH // 128             # 16 tiles of 128 rows
K_NUMER = 6                 # per-d numer matmul contraction rows
K_DIST = 8 * D + 2          # dist^2 matmul contraction rows (66)

# --- engine assignment tables (tuned against the trace) ---
# Constraints: the G-pass reads numer from PSUM and GPSIMD has no PSUM
# access, so all 8 G-stt run on DVE.  GPSIMD also rejects TensorScalarPtr
# (stt/tensor_scalar), so it can only run the plain tensor_tensor adds.
# deriv_d = t_d + G'_d with t_d = acc_d*W staged on ACT (Copy w/ scale) for
# STAGE_ACT planes and DVE tensor_scalar (4x) for the rest; the adds are
# merged tensor_tensor instructions split GPS/DVE per ADD_GROUPS.
STAGE_ACT = (0, 1, 2, 3)         # planes staged on ACT
ADD_GROUPS = (                   # (engine, first plane, last plane+1)
    ("gps", 0, 3),
    ("gps", 3, 6),
    ("dve", 6, 8),
)

_cache = {}


def _split_bf16(a):
    """fp32/fp64 array -> (hi, lo) bf16 pair with hi+lo ~ a to ~2^-16."""
    a = np.asarray(a, np.float32)
    hi = a.astype(ml_dtypes.bfloat16)
    lo = (a - hi.astype(np.float32)).astype(ml_dtypes.bfloat16)
    return hi, lo


def _force_single_act_table():
    """All activation funcs used here (ln, exp, copy, identity) live in the
    'natural_log_exp_and_others' set, but the table-load pass greedily maps
    exp/copy to set 0 and ln to set 5, thrashing two ACT_TABLE_LOADs (~2.6us)
    per tile.  Strip those funcs from every other set (ids preserved) so the
    pass resolves a single resident set for the whole kernel."""
    import concourse.bacc as _bacc
    from concourse.hw_specs import get_activation_tables as _orig

    def patched(arch):
        tabs = _orig(arch)
        keep = "natural_log_exp_and_others"
        strip = set()
        for f in ("Exp", "Ln", "Copy", "Identity", "MemsetZero", "Square",
                  "Abs", "Sign", "Relu", "Is_finite"):
            try:
                strip.add(getattr(mybir.ActivationFunctionType, f))
            except AttributeError:
                pass
        out = {}
        for name, funcs in tabs.items():
            out[name] = funcs if name == keep else (funcs - strip)
        return out

    _bacc.get_activation_tables = patched


def _build_program():
    _force_single_act_table()
    nc = bacc.Bacc("TRN2", target_bir_lowering=False, debug=False)

    xlhs_d = [nc.dram_tensor(f"xlhs{i}", [128, BSH], dt.bfloat16, kind="ExternalInput").ap()
              for i in range(4)]
    glhs_d = nc.dram_tensor("glhs", [K_DIST, BSH], dt.bfloat16, kind="ExternalInput").ap()
    hrhs_d = nc.dram_tensor("hrhs", [K_DIST, M], dt.bfloat16, kind="ExternalInput").ap()
    nrhs_d = [nc.dram_tensor(f"nrhs{i}", [128, M], dt.bfloat16, kind="ExternalInput").ap()
              for i in range(4)]
    w_d = nc.dram_tensor("w_out", [BSH, M], dt.bfloat16, kind="ExternalOutput").ap()
    dv_d = nc.dram_tensor("d_out", [BSH * D, M], dt.bfloat16, kind="ExternalOutput").ap()

    with tile.TileContext(nc) as tc:
        with tc.tile_pool(name="const", bufs=1) as cpool, \
             tc.tile_pool(name="work", bufs=3) as wpool, \
             tc.tile_pool(name="gbuf", bufs=3) as gpool, \
             tc.tile_pool(name="dbuf", bufs=2) as dpool, \
             tc.tile_pool(name="ps_s", bufs=3, space="PSUM") as ps_s, \
             tc.tile_pool(name="ps_n", bufs=5, space="PSUM") as ps_n:

            XLHS = [cpool.tile([128, BSH], dt.bfloat16, name=f"XLHS{i}", tag=f"xlhs{i}") for i in range(4)]
            GLHS = cpool.tile([K_DIST, BSH], dt.bfloat16)
            HRHS = cpool.tile([K_DIST, M], dt.bfloat16)
            NRHS = [cpool.tile([128, M], dt.bfloat16, name=f"NRHS{i}", tag=f"nrhs{i}") for i in range(4)]
            LN2 = cpool.tile([128, 1], dt.float32)
            nc.vector.memset(LN2[:], float(np.log(2.0)))
            # dist-matmul consts first: tile 0's s-matmul only needs these;
            # head slice of GLHS lands first so tile 0 starts ~2us earlier
            nc.sync.dma_start(HRHS[:], hrhs_d[:])
            nc.sync.dma_start(GLHS[:, 0:128], glhs_d[:, 0:128])
            nc.sync.dma_start(GLHS[:, 128:], glhs_d[:, 128:])
            for i in range(4):
                nc.sync.dma_start(NRHS[i][:], nrhs_d[i][:])
                nc.sync.dma_start(XLHS[i][:], xlhs_d[i][:])

            dv_t = dv_d.rearrange("(t p d) m -> t p (d m)", p=128, d=D)
            w_t = w_d.rearrange("(t p) m -> t p m", p=128)

            for t in range(NT):
                ts = slice(t * 128, (t + 1) * 128)

                # --- dist^2 via one bf16-split matmul ---
                s_ps = ps_s.tile([128, M], dt.float32, tag="s")
                nc.tensor.matmul(s_ps[:], GLHS[:, ts], HRHS[:], start=True, stop=True)

                # --- ACT chain: L = ln s ; d2 = 2*dist = exp(.5L + ln2) ;
                # U = exp(-.5*d2) (accum -> S) ; R = 1/dist = exp(-.5L)
                L = wpool.tile([128, M], dt.float32, tag="L")
                nc.scalar.activation(L[:], s_ps[:], AF.Ln)
                d2 = wpool.tile([128, M], dt.float32, tag="d2")
                nc.scalar.activation(d2[:], L[:], AF.Exp, scale=0.5, bias=LN2[:])
                U = wpool.tile([128, M], dt.bfloat16, tag="U")
                S = wpool.tile([128, 1], dt.float32, tag="S")
                nc.scalar.activation(U[:], d2[:], AF.Exp, scale=-0.5, accum_out=S[:])
                R = wpool.tile([128, M], dt.bfloat16, tag="R")
                nc.scalar.activation(R[:], L[:], AF.Exp, scale=-0.5)

                # invS = 1/S (fp32, ~2 ULP custom-DVE pair); W = U*invS on the
                # ACT engine (Copy with per-partition scale) to keep DVE free
                # for the G-passes; V = W*R on DVE (bf16 2x mode)
                invS = wpool.tile([128, 1], dt.float32, tag="invS")
                rscr = wpool.tile([128, 1], dt.float32, tag="rscr")
                nc.vector.reciprocal_approx_accurate(invS[:], S[:], rscr[:])
                W = wpool.tile([128, M], dt.bfloat16, tag="W")
                nc.scalar.activation(W[:], U[:], AF.Copy, scale=invS[:])
                V = wpool.tile([128, M], dt.bfloat16, tag="V")
                nc.vector.tensor_tensor(V[:], W[:], R[:], op=OP.mult)
                nc.sync.dma_start(w_t[t], W[:])

                # --- per-d: numer matmul ; G'_d = -numer_d*V (accum -> -acc_d) ---
                G = gpool.tile([128, D * M], dt.bfloat16, tag="G")
                nacc = wpool.tile([128, D], dt.float32, tag="nacc")
                for d in range(D):
                    np_ps = ps_n.tile([128, M], dt.float32, tag="n")
                    XL = XLHS[d // 2]
                    NRH = NRHS[d // 2]
                    p0 = 64 * (d % 2)
                    nc.tensor.matmul(
                        np_ps[:],
                        XL[p0:p0 + K_NUMER, ts],
                        NRH[p0:p0 + K_NUMER, :],
                        start=True, stop=True,
                    )
                    nc.vector.scalar_tensor_tensor(
                        G[:, d * M:(d + 1) * M], np_ps[:], -1.0, V[:],
                        op0=OP.mult, op1=OP.mult,
                        accum_out=nacc[:, d:d + 1],
                    )

                # acc = -nacc (small ACT pass; keeps DVE free)
                acc8 = wpool.tile([128, D], dt.float32, tag="acc8")
                nc.scalar.activation(acc8[:], nacc[:], AF.Copy, scale=-1.0)

                # --- deriv_d = acc_d*W + G'_d: stage t_d, then merged adds ---
                DV = dpool.tile([128, D * M], dt.bfloat16, tag="DV")
                tl = wpool.tile([128, D * M], dt.bfloat16, tag="tl")
                for d in range(D):
                    dsl = slice(d * M, (d + 1) * M)
                    if d in STAGE_ACT:
                        nc.scalar.activation(tl[:, dsl], W[:], AF.Copy,
                                             scale=acc8[:, d:d + 1])
                    else:
                        nc.vector.tensor_scalar(tl[:, dsl], W[:],
                                                acc8[:, d:d + 1], None,
                                                op0=OP.mult)
                for eng_name, d0, d1 in ADD_GROUPS:
                    gsl = slice(d0 * M, d1 * M)
                    eng = nc.gpsimd if eng_name == "gps" else nc.vector
                    eng.tensor_tensor(DV[:, gsl], tl[:, gsl], G[:, gsl],
                                      op=OP.add)

                # chunked output DMAs so the first half flies early
                nc.sync.dma_start(dv_t[t][:, 0:4 * M], DV[:, 0:4 * M])
                nc.sync.dma_start(dv_t[t][:, 4 * M:], DV[:, 4 * M:])

    nc.compile()
    return nc


def _host_prep(x, z, T):
    """Build per-core input maps.  All in fp64 for max const accuracy."""
    x64 = x.astype(np.float64)
    invT = 1.0 / T.astype(np.float64)          # [M, D]
    A = invT * invT
    Bz = z.astype(np.float64) * invT
    c = (z.astype(np.float64) ** 2).sum(axis=1)          # [M]

    Ah, Al = _split_bf16(A)                    # [M, D] each
    B2h, B2l = _split_bf16(-2.0 * Bz)
    Bnh, Bnl = _split_bf16(-Bz)
    ch, cl = _split_bf16(c)

    # dist rhs H [K_DIST, M]: groups x2h*(Ah,Al), x2l*(Ah,Al), xh*(B2h,B2l),
    # xl*(B2h,B2l), ones*(ch,cl)
    H = np.zeros((K_DIST, M), ml_dtypes.bfloat16)
    for d in range(D):
        H[0 * D + d] = Ah[:, d]
        H[1 * D + d] = Al[:, d]
        H[2 * D + d] = Ah[:, d]
        H[3 * D + d] = Al[:, d]
        H[4 * D + d] = B2h[:, d]
        H[5 * D + d] = B2l[:, d]
        H[6 * D + d] = B2h[:, d]
        H[7 * D + d] = B2l[:, d]
    H[8 * D] = ch
    H[8 * D + 1] = cl

    # numer rhs: four [128, M] tensors, d-blocks at partitions 0/64;
    # rows pair with lhsT rows [ones, ones, xh, xh, xl, xl] ->
    # [-Bzh, -Bzl, Ah, Al, Ah, Al]
    NRs = [np.zeros((128, M), ml_dtypes.bfloat16) for _ in range(4)]
    for d in range(D):
        NR = NRs[d // 2]
        b = 64 * (d % 2)
        NR[b + 0] = Bnh[:, d]
        NR[b + 1] = Bnl[:, d]
        NR[b + 2] = Ah[:, d]
        NR[b + 3] = Al[:, d]
        NR[b + 4] = Ah[:, d]
        NR[b + 5] = Al[:, d]

    in_maps = []
    for cix in range(N_CORES):
        xs = x64[cix * BSH:(cix + 1) * BSH]            # [BSH, D]
        xh, xl = _split_bf16(xs)
        x2h, x2l = _split_bf16(xs * xs)
        GL = np.zeros((K_DIST, BSH), ml_dtypes.bfloat16)
        for d in range(D):
            GL[0 * D + d] = x2h[:, d]
            GL[1 * D + d] = x2h[:, d]
            GL[2 * D + d] = x2l[:, d]
            GL[3 * D + d] = x2l[:, d]
            GL[4 * D + d] = xh[:, d]
            GL[5 * D + d] = xh[:, d]
            GL[6 * D + d] = xl[:, d]
            GL[7 * D + d] = xl[:, d]
        GL[8 * D] = 1.0
        GL[8 * D + 1] = 1.0
        XLs = [np.zeros((128, BSH), ml_dtypes.bfloat16) for _ in range(4)]
        for d in range(D):
            XL = XLs[d // 2]
            b = 64 * (d % 2)
            XL[b + 0] = 1.0
            XL[b + 1] = 1.0
            XL[b + 2] = xh[:, d]
            XL[b + 3] = xh[:, d]
            XL[b + 4] = xl[:, d]
            XL[b + 5] = xl[:, d]
        im = {"glhs": GL, "hrhs": H}
        for i in range(4):
            im[f"xlhs{i}"] = XLs[i]
            im[f"nrhs{i}"] = NRs[i]
        in_maps.append(im)
    return in_maps


def kernel(x, z, T, _trace=False):
    if "nc" not in _cache:
        _cache["nc"] = _build_program()
    nc = _cache["nc"]
    in_maps = _host_prep(np.asarray(x), np.asarray(z), np.asarray(T))
    res = run_bass_kernel_spmd(nc, in_maps, core_ids=list(range(N_CORES)), trace=_trace)
    _cache["last_exec_time_ns"] = res.exec_time_ns
    w_full = np.concatenate([r["w_out"] for r in res.results], axis=0)
    d_full = np.concatenate([r["d_out"] for r in res.results], axis=0)
    out = np.concatenate([w_full, d_full], axis=0)
    return out.astype(np.float32)
